# revision 15
# baseline (speedup 1.0000x reference)
"""Causal multi-head attention (B=4, S=2048, D=1024, H=16) on 8 NeuronCores.

Sharding: core c handles batch b=c//2 and head-group g=c%2 (8 heads, 512
features). Host pre-transposes x and the weight slices so every on-device
matmul contracts along the partition dim; per-core partial outputs of the
row-parallel out-projection are summed pairwise on the host (+ bias).

Per-core pipeline (one Bass/Tile program, SPMD over 8 cores):
  1. qT/kT = W.T-slice @ x.T (feature-major), v token-major, via float32r
     matmuls at full PE rate; results stored bf16.
  2. Flash-style causal attention per (head, 512-token q-block):
     scoresT tiles [k=128, q<=512] = kT.T @ qT, exp on ACT (scale=1/8),
     triangular-diagonal tiles masked multiplicatively, context accumulated
     as ctx_augT[65, 512] with an appended ones-column in V producing the
     softmax denominator in row 64. Fully-masked tiles are never computed.
  3. Normalize with reciprocal_approx_fast + gpsimd partition broadcast,
     write ctxT (feature-major, bf16).
  4. out_partial[2048, 1024] = ctxT.T @ Wo.T-slice (bf16), f32 out.
"""

import os
import sys
import types

import numpy as np
import ml_dtypes

import concourse.bass as bass
import concourse.mybir as mybir
from concourse import tile
from concourse.bass_utils import run_bass_kernel_spmd
from concourse.masks import make_upper_triangular

# ----------------------------------------------------------------------------
# Compat patches for this container (self-contained on purpose).
# ----------------------------------------------------------------------------


def _patch_tail_drain():
    """This walrus build accepts only ONE sync-wait per sync-engine
    instruction; TileContext's tail drain may carry several. Split extras
    onto dedicated 1-wait nops."""
    from concourse.vector_clock import ScopedClock

    def _drain_and_barrier(self, tick_clock, wait_clock):
        nc = self.nc
        drain_inst = nc.sync.drain()
        wait_clock.add_sem_waits(
            drain_inst.ins, ScopedClock({None: tick_clock.global_clock})
        )
        si = drain_inst.ins.sync_info
        if si is not None and len(si.on_wait) > 1:
            waits = list(si.on_wait)
            drain_inst.ins.sync_info = mybir.SyncInfo(
                on_wait=waits[:1], on_update=list(si.on_update)
            )
            for w in waits[1:]:
                n = nc.sync.nop()
                n.ins.sync_info = mybir.SyncInfo(on_wait=[w], on_update=[])

        nc.all_engine_barrier()
        assert self.sems is not None
        popped = nc._tile_sem_poison_stack.pop()
        assert popped is self._sem_poison
        nc.clear_and_free_semaphores(list(self.sems.allocated().values()))
        nc.all_engine_barrier()

    tile.TileContext._drain_and_barrier = _drain_and_barrier


def _patch_profiling():
    """Provide the NTFF profile hook (image's antenv lacks axon_hooks) and
    disable cloud artifact uploads. Only matters when tracing is requested."""
    import concourse.bass_utils as bass_utils

    bass_utils.upload_artifacts = lambda tmpdir: tmpdir
    try:
        from antenv.axon_hooks import get_axon_ntff_profile_hook  # noqa: F401
        return
    except ImportError:
        pass
    try:
        from trn_agent_boot.trn_boot import _ntff_profile_via_ctypes

        hook = _ntff_profile_via_ctypes("/opt/axon/libaxon_pjrt.so")
    except Exception:
        hook = None
    mod = types.ModuleType("antenv.axon_hooks")
    mod._hook = hook
    mod.get_axon_ntff_profile_hook = lambda: mod._hook
    mod.set_axon_ntff_profile_hook = lambda h: setattr(mod, "_hook", h)
    sys.modules["antenv.axon_hooks"] = mod
    import antenv

    antenv.axon_hooks = mod


_patch_tail_drain()
_patch_profiling()


def _legalize_waits(nc):
    """This walrus build allows 1 sync-wait per instruction (2 on
    EventSemaphore). Split excess waits onto EventSemaphore carriers
    inserted just before the over-capacity instruction (same engine
    queue, so ordering semantics are preserved)."""
    n_fix = 0
    for f in nc.m.functions:
        for b in f.blocks:
            out = []
            changed = False
            for inst in b.instructions:
                si = inst.sync_info
                cap = 1
                if si is not None and len(si.on_wait) > cap:
                    waits = list(si.on_wait)
                    extra, keep = waits[:-cap], waits[-cap:]
                    for i in range(0, len(extra), 1):
                        n_fix += 1
                        out.append(
                            mybir.InstNoOp(
                                name=f"I-waitfix-{n_fix}",
                                engine=inst.engine,
                                ins=[],
                                outs=[],
                                sync_info=mybir.SyncInfo(
                                    on_wait=extra[i:i + 1], on_update=[]
                                ),
                            )
                        )
                    inst.sync_info = mybir.SyncInfo(
                        on_wait=keep, on_update=list(si.on_update)
                    )
                    changed = True
                out.append(inst)
            if changed:
                b.instructions = out

# ----------------------------------------------------------------------------
# Problem constants (hardcoded; kernel.py must be self-contained).
# ----------------------------------------------------------------------------
B, S, D, H = 4, 2048, 1024, 16
HD = D // H          # 64 head dim
NCORES = 8
GPC = 2              # head-groups per batch (cores per batch)
FPC = D // GPC       # 512 features per core
HPC = H // GPC       # 8 heads per core
P = 128
DC = D // P          # 8 contraction chunks
NT = S // P          # 16 token tiles
QB = 512             # q-block
NQB = S // QB        # 4

F32 = mybir.dt.float32
F32R = mybir.dt.float32r
BF16 = mybir.dt.bfloat16
EXPF = mybir.ActivationFunctionType.Exp
SCALE = 1.0 / np.sqrt(HD)


def _build_program():
    nc = bass.Bass("TRN2", target_bir_lowering=False, debug=False, num_devices=1)
    xT = nc.dram_tensor("xT", [D, S], F32R, kind="ExternalInput").ap()
    wq = nc.dram_tensor("wq", [D, FPC], F32R, kind="ExternalInput").ap()
    wk = nc.dram_tensor("wk", [D, FPC], F32R, kind="ExternalInput").ap()
    wv = nc.dram_tensor("wv", [D, FPC], F32R, kind="ExternalInput").ap()
    wo = nc.dram_tensor("wo", [FPC, D], BF16, kind="ExternalInput").ap()
    out = nc.dram_tensor("out", [S, D], F32, kind="ExternalOutput").ap()

    with tile.TileContext(nc) as tc:
        _emit(nc, tc, xT, wq, wk, wv, wo, out)
    _legalize_waits(nc)
    return nc


def _emit(nc, tc, xT, wq, wk, wv, wo, out):
    persist = tc.alloc_tile_pool(name="persist", bufs=1)

    qT = persist.tile([P, NQB, S], BF16, tag="qT")
    kT = persist.tile([P, NQB, S], BF16, tag="kT")
    vtm = persist.tile([P, NT, HPC, HD + 1], BF16, tag="vtm")
    ctxT = persist.tile([P, NQB, S], BF16, tag="ctxT")
    wo_sb = persist.tile([P, FPC // P, D], BF16, tag="wo_sb")
    dmask_f = persist.tile([P, P], F32, tag="dmask_f")
    dmask = persist.tile([P, P], BF16, tag="dmask")

    # one-time setup
    nc.sync.dma_start(wo_sb[:], wo.rearrange("(c p) e -> p c e", p=P))
    make_upper_triangular(nc, dmask_f[:], val=1.0, diag=True)
    nc.vector.tensor_copy(dmask[:], dmask_f[:])
    nc.vector.memset(vtm[:], 1.0)  # ones column; v slots overwritten below

    # ---------------- Phase A: QKV projections (float32r) ----------------
    with (
        tc.tile_pool(name="loadA", bufs=1) as la,
        tc.tile_pool(name="psA", bufs=6, space="PSUM") as psA,
    ):
        xT_sb = la.tile([P, DC, S], F32R, tag="xT_sb")
        wq_sb = la.tile([P, DC, FPC], F32R, tag="wq_sb")
        wk_sb = la.tile([P, DC, FPC], F32R, tag="wk_sb")
        wv_sb = la.tile([P, DC, FPC], F32R, tag="wv_sb")
        nc.sync.dma_start(xT_sb[:], xT.rearrange("(c p) n -> p c n", p=P))
        nc.sync.dma_start(wq_sb[:], wq.rearrange("(c p) e -> p c e", p=P))
        nc.sync.dma_start(wk_sb[:], wk.rearrange("(c p) e -> p c e", p=P))
        nc.sync.dma_start(wv_sb[:], wv.rearrange("(c p) e -> p c e", p=P))

        for w_sb, dst in ((wq_sb, qT), (wk_sb, kT)):
            for m in range(FPC // P):
                for nb in range(NQB):
                    ps = psA.tile([P, QB], F32, tag="psA")
                    for dk in range(DC):
                        nc.tensor.matmul(
                            ps[:],
                            lhsT=w_sb[:, dk, m * P:(m + 1) * P],
                            rhs=xT_sb[:, dk, nb * QB:(nb + 1) * QB],
                            start=(dk == 0),
                            stop=(dk == DC - 1),
                        )
                    nc.vector.tensor_copy(dst[:, m, nb * QB:(nb + 1) * QB], ps[:])

        for nt in range(NT):
            ps = psA.tile([P, FPC], F32, tag="psA")
            for dk in range(DC):
                nc.tensor.matmul(
                    ps[:],
                    lhsT=xT_sb[:, dk, nt * P:(nt + 1) * P],
                    rhs=wv_sb[:, dk, :],
                    start=(dk == 0),
                    stop=(dk == DC - 1),
                )
            nc.vector.tensor_copy(
                vtm[:, nt, :, 0:HD],
                ps[:].rearrange("p (h d) -> p h d", h=HPC),
            )

    # ---------------- Phase B: causal attention ----------------
    # Unnormalized ctx rows staged f32 in ctxU; denominators staged via a
    # p64 hop + tiny DMA into denstage rows; one batched reciprocal at the
    # end, broadcast per (head, q-block) with a K=1 PE outer product.
    stageB = tc.alloc_tile_pool(name="stageB", bufs=1)
    ctxU = stageB.tile([P, NQB, S], F32, tag="ctxU")
    denstage = stageB.tile([HPC * NQB, QB], F32, tag="denstage")
    recstage = stageB.tile([HPC * NQB, QB], F32, tag="recstage")
    ones_sb = stageB.tile([P, HD], F32, tag="ones_sb")
    nc.vector.memset(ones_sb[:], 1.0)

    with (
        tc.tile_pool(name="expp", bufs=6) as expp,
        tc.tile_pool(name="scps", bufs=2, space="PSUM") as scps,
        tc.tile_pool(name="ctxps", bufs=2, space="PSUM") as ctxps,
        tc.tile_pool(name="dtmpp", bufs=3) as dtmpp,
    ):
        for h in range(HPC):
            m2 = h // 2
            hp = (h % 2) * HD
            kslice = lambda kt: kT[hp:hp + HD, m2, kt * P:(kt + 1) * P]
            for qb in range(NQB):
                qs = lambda q0: qT[hp:hp + HD, m2, qb * QB + q0:(qb + 1) * QB]
                pctx = ctxps.tile([HD + 1, QB], F32, tag="pctx")

                # group list: full k-tiles in 3s, then the diagonal group
                fulls = list(range(4 * qb))
                groups = [("full", fulls[i:i + 3]) for i in range(0, len(fulls), 3)]
                groups.append(("diag", [4 * qb + j for j in range(4)]))

                pending = None  # (kind, kts, es) awaiting ctx matmuls
                first_ctx = True

                def emit_ctx(kind, kts, es, last_group):
                    nonlocal first_ctx
                    if kind == "full":
                        for i, kt in enumerate(kts):
                            nc.tensor.matmul(
                                pctx[:],
                                lhsT=vtm[:, kt, h, :],
                                rhs=es[:, i * QB:i * QB + QB],
                                start=first_ctx,
                                stop=False,
                                skip_group_check=True,
                            )
                            first_ctx = False
                    else:
                        offs = (0, 512, 1024, 1280)
                        lens = (512, 384, 256, 128)
                        qoffs = (0, 128, 256, 384)
                        for j in range(4):
                            nc.tensor.matmul(
                                pctx[:, qoffs[j]:QB],
                                lhsT=vtm[:, kts[j], h, :],
                                rhs=es[:, offs[j]:offs[j] + lens[j]],
                                start=first_ctx,
                                stop=(j == 3),
                                skip_group_check=True,
                            )
                            first_ctx = False

                for kind, kts in groups:
                    ps = scps.tile([P, 3 * QB], F32, tag="scps")
                    es = expp.tile([P, 3 * QB], BF16, tag="es")
                    if kind == "full":
                        for i, kt in enumerate(kts):
                            nc.tensor.matmul(
                                ps[:, i * QB:i * QB + QB],
                                lhsT=kslice(kt),
                                rhs=qs(0),
                                start=True,
                                stop=True,
                            )
                        n = len(kts) * QB
                        nc.scalar.activation(es[:, 0:n], ps[:, 0:n], EXPF, scale=SCALE)
                    else:
                        offs = (0, 512, 1024, 1280)
                        lens = (512, 384, 256, 128)
                        qoffs = (0, 128, 256, 384)
                        for j in range(4):
                            nc.tensor.matmul(
                                ps[:, offs[j]:offs[j] + lens[j]],
                                lhsT=kslice(kts[j]),
                                rhs=qs(qoffs[j]),
                                start=(j != 3),
                                stop=(j != 2),
                                skip_group_check=True,
                            )
                        nc.scalar.activation(
                            es[:, 0:896], ps[:, 0:896], EXPF, scale=SCALE
                        )
                        nc.scalar.activation(
                            es[:, 1024:1408], ps[:, 1024:1408], EXPF, scale=SCALE
                        )
                        for j in range(4):
                            o = offs[j]
                            nc.vector.tensor_mul(
                                es[:, o:o + P], es[:, o:o + P], dmask[:]
                            )
                    if pending is not None:
                        emit_ctx(pending[0], pending[1], pending[2], False)
                    pending = (kind, kts, es)
                emit_ctx(pending[0], pending[1], pending[2], True)

                # stage unnormalized ctx (rows 0..63) and denominator row
                nc.vector.tensor_copy(
                    ctxU[hp:hp + HD, m2, qb * QB:(qb + 1) * QB], pctx[0:HD, :]
                )
                dtmp = dtmpp.tile([P, QB], F32, tag="dtmp")
                nc.vector.tensor_copy(dtmp[HD:HD + 1, :], pctx[HD:HD + 1, :])
                idx = h * NQB + qb
                nc.sync.dma_start(denstage[idx:idx + 1, :], dtmp[HD:HD + 1, :])

    # ---------------- Phase B2: batched softmax normalization ----------------
    nc.vector.reciprocal(recstage[:], denstage[:])
    with (
        tc.tile_pool(name="normps", bufs=2, space="PSUM") as normps,
        tc.tile_pool(name="rrowp", bufs=4) as rrowp,
    ):
        for m2 in range(NQB):
            for qb in range(NQB):
                bcps = normps.tile([P, QB], F32, tag="bcps")
                for half in range(2):
                    idx = (2 * m2 + half) * NQB + qb
                    rrow = rrowp.tile([P, QB], F32, tag="rrow")
                    nc.sync.dma_start(rrow[HD:HD + 1, :], recstage[idx:idx + 1, :])
                    nc.tensor.matmul(
                        bcps[half * HD:(half + 1) * HD, :],
                        lhsT=ones_sb[HD:HD + 1, :],
                        rhs=rrow[HD:HD + 1, :],
                        start=True,
                        stop=True,
                    )
                nc.vector.tensor_mul(
                    ctxT[:, m2, qb * QB:(qb + 1) * QB],
                    ctxU[:, m2, qb * QB:(qb + 1) * QB],
                    bcps[:],
                )

    stageB.release()

    # ---------------- Phase C: out-projection ----------------
    with (
        tc.tile_pool(name="outps", bufs=4, space="PSUM") as outps,
        tc.tile_pool(name="outsb", bufs=3) as outsb,
    ):
        for nt in range(NT):
            stage = outsb.tile([P, D], F32, tag="stage")
            for ec in range(D // QB):
                ps = outps.tile([P, QB], F32, tag="outps")
                for m in range(FPC // P):
                    nc.tensor.matmul(
                        ps[:],
                        lhsT=ctxT[:, m, nt * P:(nt + 1) * P],
                        rhs=wo_sb[:, m, ec * QB:(ec + 1) * QB],
                        start=(m == 0),
                        stop=(m == FPC // P - 1),
                    )
                nc.vector.tensor_copy(stage[:, ec * QB:(ec + 1) * QB], ps[:])
            nc.sync.dma_start(out[nt * P:(nt + 1) * P, :], stage[:])

    persist.release()


_program_cache = None
last_results = None


def _get_program():
    global _program_cache
    if _program_cache is None:
        _program_cache = _build_program()
    return _program_cache


def kernel(x, Wq, Wk, Wv, Wo, bo):
    global last_results
    x = np.asarray(x, dtype=np.float32)
    Wq = np.asarray(Wq, dtype=np.float32)
    Wk = np.asarray(Wk, dtype=np.float32)
    Wv = np.asarray(Wv, dtype=np.float32)
    Wo = np.asarray(Wo, dtype=np.float32)
    bo = np.asarray(bo, dtype=np.float32)

    in_maps = []
    for c in range(NCORES):
        b, g = c // GPC, c % GPC
        fs = slice(g * FPC, (g + 1) * FPC)
        in_maps.append(
            {
                "xT": np.ascontiguousarray(x[b].T),
                "wq": np.ascontiguousarray(Wq[fs, :].T),
                "wk": np.ascontiguousarray(Wk[fs, :].T),
                "wv": np.ascontiguousarray(Wv[fs, :].T),
                "wo": np.ascontiguousarray(Wo[:, fs].T).astype(ml_dtypes.bfloat16),
            }
        )

    nc = _get_program()
    res = run_bass_kernel_spmd(nc, in_maps, core_ids=list(range(NCORES)))
    last_results = res

    outf = np.empty((B, S, D), dtype=np.float32)
    for b in range(B):
        outf[b] = res.results[GPC * b]["out"] + res.results[GPC * b + 1]["out"] + bo
    return outf
